# revision 11
# baseline (speedup 1.0000x reference)
"""Trainium2 Bass kernel for CustomLSTM (B=32, S=1024, I=H=512).

Strategy (data-parallel over batch, per the sharding hint):
  - 8 cores, each owns 4 batch elements end-to-end. No collectives.
  - Phase 1 (per core): xW^T = W^T @ x^T + bias for all timesteps, computed
    transposed (gate-cols on partitions) so phase 2 consumes it directly.
    Output staged to a per-core DRAM scratch laid out slab-by-slab.
  - Phase 2 (per core): the sequential recurrence. Per step:
      gates^T[2048, 4] = U^T h_{t-1}^T   (PE: U stationary bf16 (FWL),
                                          h moving bf16; 16 m-tiles x 4 k)
      += xw_t^T                          (DVE add, PSUM + SBUF slab -> SBUF)
      i,f,o = sigmoid, g = tanh          (ACT; gate cols host-permuted to
                                          [i|f|o|g] so one sigmoid covers 3)
      c = f*c + i*g; h = o*tanh(c)       (DVE/ACT on [128,16] tiles)
    h tiles live in a per-slab SBUF ring that doubles as the hidden-state
    history; one DMA per slab dumps it to the output.

  Host side does layout only: transposes/permutes/casts inputs, and
  reassembles outputs.
"""

import sys

sys.path.insert(0, "/opt/trn_rl_repo")

import numpy as np
import ml_dtypes
from contextlib import ExitStack

import concourse.bass as bass
import concourse.tile as tile
from concourse import bacc, mybir
from concourse import bass_utils

BF16 = mybir.dt.bfloat16
F32 = mybir.dt.float32

B, S, I, H = 32, 1024, 512, 512
G4 = 4 * H  # 2048 gate columns
NCORES = 8
BC = B // NCORES  # 4 batches per core

SLAB = 64  # timesteps per slab (For_i body)
NSLAB = S // SLAB
KT = I // 128  # 4 k-tiles for the I/H contraction
MT = G4 // 128  # 16 m-tiles of gate columns
NCOLS = S * BC  # 4096 phase-1 moving columns (s-major, b-minor)
PSN = 512  # phase-1 psum free size
NCHUNK = NCOLS // PSN  # 8
SLABC = SLAB * BC  # 256 cols per slab

PROFILE = False  # set by test.py to request timing reruns
LAST_RESULTS = {}  # test.py introspection

U_FP8 = True  # U stationary in fp8e4m3 (scaled by USCALE); else bf16
USCALE = 512.0
GATE_SPLIT = True  # per-gate psum tiles + gate-ordered sweep to hide EW
FP8 = mybir.dt.float8e4


def _gate_perm():
    # reference gate order is [i, f, g, o]; we want [i, f, o, g]
    idx = np.arange(G4).reshape(4, H)
    return np.concatenate([idx[0], idx[1], idx[3], idx[2]])


def build_nc(steps=S):
    nslab = steps // SLAB
    ncols = steps * BC
    nchunk = max(1, ncols // PSN)
    psn = ncols // nchunk
    pieces = psn // SLABC  # slab-pieces per psum chunk

    nc = bacc.Bacc("TRN2", target_bir_lowering=False, debug=False)

    xT = nc.dram_tensor("xT", [I, ncols], BF16, kind="ExternalInput")
    Wp = nc.dram_tensor("Wp", [I, G4], BF16, kind="ExternalInput")
    Up = nc.dram_tensor("Up", [H, G4], FP8 if U_FP8 else BF16,
                        kind="ExternalInput")
    biasT = nc.dram_tensor("biasT", [128, MT], F32, kind="ExternalInput")
    hs = nc.dram_tensor("hs", [128, nslab, SLAB * KT * BC], BF16,
                        kind="ExternalOutput")
    cT = nc.dram_tensor("cT", [128, KT * BC], F32, kind="ExternalOutput")
    # per-slab staging of xw^T: [slab, m-tile, partition, slab-cols]
    xwT = nc.dram_tensor("xwT", [nslab, MT, 128, SLABC], BF16, kind="Internal")

    with TileBuild(nc, steps, nslab, nchunk, psn, pieces,
                   xT, Wp, Up, biasT, hs, cT, xwT) as _:
        pass
    nc.compile()
    return nc


class TileBuild:
    def __init__(self, nc, steps, nslab, nchunk, psn, pieces,
                 xT, Wp, Up, biasT, hs, cT, xwT):
        self.args = (nc, steps, nslab, nchunk, psn, pieces,
                     xT, Wp, Up, biasT, hs, cT, xwT)

    def __enter__(self):
        (nc, steps, nslab, nchunk, psn, pieces,
         xT, Wp, Up, biasT, hs, cT, xwT) = self.args
        with tile.TileContext(nc) as tc:
            with ExitStack() as ctx:
                self.build(ctx, tc, nc, steps, nslab, nchunk, psn, pieces,
                           xT, Wp, Up, biasT, hs, cT, xwT)
        return self

    def __exit__(self, *a):
        return False

    @staticmethod
    def build(ctx, tc, nc, steps, nslab, nchunk, psn, pieces,
              xT, Wp, Up, biasT, hs, cT, xwT):
        ncols = steps * BC

        const = ctx.enter_context(tc.tile_pool(name="const", bufs=1))

        # ---- resident constants ----
        U_sb = const.tile([128, KT, G4], FP8 if U_FP8 else BF16, tag="U_sb")
        nc.sync.dma_start(U_sb[:], Up.ap().rearrange("(k p) g -> p k g", p=128))
        bias_sb = const.tile([128, MT], F32, tag="bias_sb")
        nc.sync.dma_start(bias_sb[:], biasT.ap())

        # ---- phase 1: xw^T = W^T x^T + bias ----
        with (
            tc.tile_pool(name="p1sbuf", bufs=1) as p1s,
            tc.tile_pool(name="p1psum", bufs=4, space="PSUM") as p1p,
            tc.tile_pool(name="p1out", bufs=4) as p1o,
        ):
            W_sb = p1s.tile([128, KT, G4], BF16, tag="W_sb")
            nc.sync.dma_start(W_sb[:], Wp.ap().rearrange("(k p) g -> p k g", p=128))
            xT_sb = p1s.tile([128, KT, ncols], BF16, tag="xT_sb")
            nc.sync.dma_start(xT_sb[:], xT.ap().rearrange("(k p) c -> p k c", p=128))

            for m in range(MT):
                for n in range(nchunk):
                    ps = p1p.tile([128, psn], F32, tag="p1ps")
                    for k in range(KT):
                        nc.tensor.matmul(
                            ps[:],
                            W_sb[:, k, 128 * m:128 * (m + 1)],
                            xT_sb[:, k, psn * n:psn * (n + 1)],
                            start=(k == 0),
                            stop=(k == KT - 1),
                        )
                    st = p1o.tile([128, psn], BF16, tag="p1st")
                    nc.scalar.activation(
                        st[:], ps[:],
                        mybir.ActivationFunctionType.Identity,
                        bias=bias_sb[:, m:m + 1],
                    )
                    for q in range(pieces):
                        nc.sync.dma_start(
                            xwT.ap()[n * pieces + q, m, :, :],
                            st[:, SLABC * q:SLABC * (q + 1)],
                        )

        # ---- phase 2: recurrence ----
        state = ctx.enter_context(tc.tile_pool(name="state", bufs=1))
        c_sb = state.tile([128, KT * BC], F32, tag="c_sb")
        nc.vector.memset(c_sb[:], 0.0)
        h0 = state.tile([128, KT * BC], BF16, tag="h0")
        nc.vector.memset(h0[:], 0.0)

        slab_pool = ctx.enter_context(tc.tile_pool(name="slab", bufs=2))
        stage_pool = ctx.enter_context(tc.tile_pool(name="stage", bufs=2))
        ew_pool = ctx.enter_context(tc.tile_pool(name="ew", bufs=3))
        ps_pool = ctx.enter_context(tc.tile_pool(name="ps2", bufs=2, space="PSUM"))

        GW = KT * BC  # 16 free cols per step of h/c layout

        def loop_body(i):
            # xw slab for this iteration: [128, MT * SLABC] bf16
            slab = slab_pool.tile([128, MT, SLABC], BF16, tag="slab")
            nc.sync.dma_start(
                slab[:],
                xwT.ap()[bass.ds(i, 1)].rearrange("one m p c -> p (one m) c"),
            )
            # hidden-state ring for this slab; doubles as DMA staging
            stg = stage_pool.tile([128, SLAB * GW], BF16, tag="stg")

            Sig = mybir.ActivationFunctionType.Sigmoid
            Tanh = mybir.ActivationFunctionType.Tanh

            def mm_gate(gate, h_prev, step_tag):
                """4 m-tiles x KT k-tiles for one gate -> [128, GW] psum."""
                psg = ps_pool.tile([128, GW], F32, tag=f"ps_{gate}")
                for mi in range(4):
                    m = 4 * gate + mi
                    for k in range(KT):
                        nc.tensor.matmul(
                            psg[:, BC * mi:BC * (mi + 1)],
                            U_sb[:, k, 128 * m:128 * (m + 1)],
                            h_prev[:, BC * k:BC * (k + 1)],
                            start=(k == 0),
                            stop=(k == KT - 1),
                        )
                return psg

            def gates_add(psg, gate, s, out_tag):
                """gates^T = psum (optionally /USCALE) + xw_t^T -> fp32."""
                gg = ew_pool.tile([128, 4, BC], F32, tag=out_tag)
                xw = slab[:, 4 * gate:4 * (gate + 1), BC * s:BC * (s + 1)]
                ps3 = psg[:].rearrange("p (m b) -> p m b", m=4)
                if U_FP8:
                    nc.vector.scalar_tensor_tensor(
                        gg[:], ps3, 1.0 / USCALE, xw,
                        op0=mybir.AluOpType.mult, op1=mybir.AluOpType.add)
                else:
                    nc.vector.tensor_tensor(
                        gg[:], ps3, xw, op=mybir.AluOpType.add)
                return gg[:].rearrange("p m b -> p (m b)")

            for s in range(SLAB):
                h_prev = h0[:] if s == 0 else stg[:, (s - 1) * GW:s * GW]
                if GATE_SPLIT:
                    # host gate order [i|f|o|g]: tiles i=0:4 f=4:8 o=8:12 g=12:16
                    # sweep g, i, f, o so the EW chain overlaps the PE sweep
                    ps_g = mm_gate(3, h_prev, s)
                    ps_i = mm_gate(0, h_prev, s)
                    ps_f = mm_gate(1, h_prev, s)
                    ps_o = mm_gate(2, h_prev, s)
                    gv = gates_add(ps_g, 3, s, "gg_g")
                    gt = ew_pool.tile([128, GW], BF16, tag="gt")
                    nc.scalar.activation(gt[:], gv, Tanh)
                    iv = gates_add(ps_i, 0, s, "gg_i")
                    si = ew_pool.tile([128, GW], BF16, tag="si")
                    nc.scalar.activation(si[:], iv, Sig)
                    t_ig = ew_pool.tile([128, GW], F32, tag="t_ig")
                    nc.vector.tensor_mul(t_ig[:], si[:], gt[:])
                    fv = gates_add(ps_f, 1, s, "gg_f")
                    sf = ew_pool.tile([128, GW], BF16, tag="sf")
                    nc.scalar.activation(sf[:], fv, Sig)
                    t_fc = ew_pool.tile([128, GW], F32, tag="t_fc")
                    nc.vector.tensor_mul(t_fc[:], sf[:], c_sb[:])
                    nc.vector.tensor_add(c_sb[:], t_fc[:], t_ig[:])
                    tc_t = ew_pool.tile([128, GW], BF16, tag="tc_t")
                    nc.scalar.activation(tc_t[:], c_sb[:], Tanh)
                    ov = gates_add(ps_o, 2, s, "gg_o")
                    so = ew_pool.tile([128, GW], BF16, tag="so")
                    nc.scalar.activation(so[:], ov, Sig)
                    nc.vector.tensor_mul(stg[:, s * GW:(s + 1) * GW],
                                         so[:], tc_t[:])
                else:
                    ps = ps_pool.tile([128, MT * BC], F32, tag="gps")
                    for m in range(MT):
                        for k in range(KT):
                            nc.tensor.matmul(
                                ps[:, BC * m:BC * (m + 1)],
                                U_sb[:, k, 128 * m:128 * (m + 1)],
                                h_prev[:, BC * k:BC * (k + 1)],
                                start=(k == 0),
                                stop=(k == KT - 1),
                            )
                    g_sb = ew_pool.tile([128, MT * BC], F32, tag="g_sb")
                    if U_FP8:
                        nc.vector.scalar_tensor_tensor(
                            g_sb[:].rearrange("p (m b) -> p m b", m=MT),
                            ps[:].rearrange("p (m b) -> p m b", m=MT),
                            1.0 / USCALE,
                            slab[:, :, BC * s:BC * (s + 1)],
                            op0=mybir.AluOpType.mult, op1=mybir.AluOpType.add)
                    else:
                        nc.vector.tensor_add(
                            g_sb[:], ps[:], slab[:, :, BC * s:BC * (s + 1)])
                    sif = ew_pool.tile([128, 3 * GW], BF16, tag="sif")
                    nc.scalar.activation(sif[:], g_sb[:, 0:3 * GW], Sig)
                    gt = ew_pool.tile([128, GW], BF16, tag="gt")
                    nc.scalar.activation(gt[:], g_sb[:, 3 * GW:4 * GW], Tanh)
                    t_fc = ew_pool.tile([128, GW], F32, tag="t_fc")
                    nc.vector.tensor_mul(t_fc[:], sif[:, GW:2 * GW], c_sb[:])
                    t_ig = ew_pool.tile([128, GW], F32, tag="t_ig")
                    nc.vector.tensor_mul(t_ig[:], sif[:, 0:GW], gt[:])
                    nc.vector.tensor_add(c_sb[:], t_fc[:], t_ig[:])
                    tc_t = ew_pool.tile([128, GW], BF16, tag="tc_t")
                    nc.scalar.activation(tc_t[:], c_sb[:], Tanh)
                    nc.vector.tensor_mul(stg[:, s * GW:(s + 1) * GW],
                                         sif[:, 2 * GW:3 * GW], tc_t[:])

            # persist last h for the next slab, dump the slab's hidden states
            nc.vector.tensor_copy(h0[:], stg[:, (SLAB - 1) * GW:SLAB * GW])
            nc.sync.dma_start(hs.ap()[:, bass.ds(i, 1), :],
                              stg[:].rearrange("p (one c) -> p one c", one=1))

        if nslab == 1:
            loop_body(nc.snap(0))
        else:
            with tc.For_i(0, nslab) as i:
                loop_body(i)

        nc.sync.dma_start(cT.ap()[:, :], c_sb[:])


def _prep_core_inputs(x, W, U, bias):
    """Host-side layout prep shared across cores (W/U/bias) + per-core xT."""
    perm = _gate_perm()
    steps = x.shape[1]
    Wp = np.ascontiguousarray(W[:, perm]).astype(ml_dtypes.bfloat16)
    if U_FP8:
        Up = np.ascontiguousarray(U[:, perm] * USCALE).astype(
            mybir.dt.np(FP8))
    else:
        Up = np.ascontiguousarray(U[:, perm]).astype(ml_dtypes.bfloat16)
    bp = bias[perm].astype(np.float32)
    biasT = np.ascontiguousarray(bp.reshape(MT, 128).T)  # [128, MT]
    in_maps = []
    for c in range(NCORES):
        xc = x[c * BC:(c + 1) * BC]  # [BC, steps, I]
        xTc = np.ascontiguousarray(
            np.transpose(xc, (2, 1, 0)).reshape(I, steps * BC)
        ).astype(ml_dtypes.bfloat16)
        in_maps.append({"xT": xTc, "Wp": Wp, "Up": Up, "biasT": biasT})
    return in_maps


def _assemble(results):
    """Per-core outputs -> full (hidden_seq, h_T, c_T)."""
    hs_parts, cT_parts = [], []
    for r in results:
        hsd = np.asarray(r["hs"]).astype(np.float32)  # [128, NSLAB, SLAB*16]
        # hs[p, l, (s, j, b)] with unit = 128*j + p
        hsd = hsd.reshape(128, NSLAB, SLAB, KT, BC)
        # -> [b, t, unit] = [b, l*SLAB+s, j*128+p]
        hsd = np.transpose(hsd, (4, 1, 2, 3, 0)).reshape(BC, S, H)
        hs_parts.append(hsd)
        ctd = np.asarray(r["cT"]).astype(np.float32).reshape(128, KT, BC)
        cT_parts.append(np.transpose(ctd, (2, 1, 0)).reshape(BC, H))
    hidden = np.concatenate(hs_parts, axis=0)
    c_T = np.concatenate(cT_parts, axis=0)
    h_T = hidden[:, -1, :].copy()
    return hidden, h_T, c_T


def _run_spmd(nc, in_maps, n_timed=0):
    """Execute the bass program on NCORES cores via PJRT (axon).

    Mirrors bass2jax.run_bass_via_pjrt's multi-core tail, but keeps the
    jitted callable so optional timing reruns skip recompilation.
    """
    import jax
    import time
    from jax.sharding import Mesh, PartitionSpec
    from jax.experimental.shard_map import shard_map
    from concourse import bass2jax, mybir as _mybir
    from concourse.bass2jax import (
        _bass_exec_p, install_neuronx_cc_hook, partition_id_tensor)

    install_neuronx_cc_hook()

    partition_name = (nc.partition_id_tensor.name
                      if nc.partition_id_tensor else None)
    in_names, out_names, out_avals, zero_outs = [], [], [], []
    for alloc in nc.m.functions[0].allocations:
        if not isinstance(alloc, _mybir.MemoryLocationSet):
            continue
        name = alloc.memorylocations[0].name
        if alloc.kind == "ExternalInput":
            if name != partition_name:
                in_names.append(name)
        elif alloc.kind == "ExternalOutput":
            shape = tuple(alloc.tensor_shape)
            dtype = _mybir.dt.np(alloc.dtype)
            out_names.append(name)
            out_avals.append(jax.core.ShapedArray(shape, dtype))
            zero_outs.append(np.zeros(shape, dtype))
    n_params = len(in_names)
    n_outs = len(out_avals)
    all_names = in_names + out_names
    if partition_name is not None:
        all_names.append(partition_name)

    def _body(*args):
        operands = list(args)
        if partition_name is not None:
            operands.append(partition_id_tensor())
        outs = _bass_exec_p.bind(
            *operands,
            out_avals=tuple(out_avals),
            in_names=tuple(all_names),
            out_names=tuple(out_names),
            lowering_input_output_aliases=(),
            sim_require_finite=True,
            sim_require_nnan=True,
            nc=nc,
        )
        return tuple(outs)

    devices = jax.devices()[:NCORES]
    mesh = Mesh(np.asarray(devices), ("core",))
    sharded = jax.jit(
        shard_map(
            _body, mesh=mesh,
            in_specs=(PartitionSpec("core"),) * (n_params + n_outs),
            out_specs=(PartitionSpec("core"),) * n_outs,
            check_rep=False,
        ),
        donate_argnums=tuple(range(n_params, n_params + n_outs)),
        keep_unused=True,
    )
    concat_in = [
        np.concatenate([np.asarray(m[name]) for m in in_maps], axis=0)
        for name in in_names
    ]
    # keep inputs resident on device so timing reruns measure execution only
    sharding = jax.sharding.NamedSharding(mesh, PartitionSpec("core"))
    concat_in = [jax.device_put(a, sharding) for a in concat_in]

    def once():
        zs = [np.zeros((NCORES * z.shape[0], *z.shape[1:]), z.dtype)
              for z in zero_outs]
        out = sharded(*concat_in, *zs)
        jax.block_until_ready(out)
        return out

    out_arrs = once()
    times = []
    for _ in range(n_timed):
        t0 = time.perf_counter()
        once()
        times.append((time.perf_counter() - t0) * 1e9)
    results = [
        {name: np.asarray(out_arrs[i]).reshape(NCORES, *out_avals[i].shape)[c]
         for i, name in enumerate(out_names)}
        for c in range(NCORES)
    ]
    return results, (min(times) if times else None)


def kernel(x, W, U, bias):
    x = np.asarray(x, dtype=np.float32)
    W = np.asarray(W, dtype=np.float32)
    U = np.asarray(U, dtype=np.float32)
    bias = np.asarray(bias, dtype=np.float32)

    nc = build_nc(S)
    in_maps = _prep_core_inputs(x, W, U, bias)
    results, t_ns = _run_spmd(nc, in_maps, n_timed=3 if PROFILE else 0)
    LAST_RESULTS["exec_time_ns"] = t_ns
    return _assemble(results)


if __name__ == "__main__":
    # smoke test in CoreSim with a short sequence
    from concourse.bass_interp import CoreSim

    steps = 64
    nc = build_nc(steps)
    rng = np.random.default_rng(0)
    stdv = 1.0 / np.sqrt(H)
    x = rng.standard_normal((B, steps, I), dtype=np.float32)
    W = rng.uniform(-stdv, stdv, (I, G4)).astype(np.float32)
    U = rng.uniform(-stdv, stdv, (H, G4)).astype(np.float32)
    bias = rng.uniform(-stdv, stdv, (G4,)).astype(np.float32)

    # numpy reference
    def ref(x, W, U, bias):
        b, s, _ = x.shape
        xW = np.einsum("bsi,ig->bsg", x, W) + bias
        h = np.zeros((b, H), np.float32)
        c = np.zeros((b, H), np.float32)
        hsout = np.zeros((b, s, H), np.float32)
        for t in range(s):
            gates = xW[:, t] + h @ U
            i_t = 1 / (1 + np.exp(-gates[:, :H]))
            f_t = 1 / (1 + np.exp(-gates[:, H:2 * H]))
            g_t = np.tanh(gates[:, 2 * H:3 * H])
            o_t = 1 / (1 + np.exp(-gates[:, 3 * H:]))
            c = f_t * c + i_t * g_t
            h = o_t * np.tanh(c)
            hsout[:, t] = h
        return hsout, h, c

    exp_hs, exp_h, exp_c = ref(x, W, U, bias)

    # simulate core 0 only
    global S_orig
    in_maps = _prep_core_inputs(x[:, :steps], W, U, bias)
    sim = CoreSim(nc)
    for k, v in in_maps[0].items():
        sim.tensor(k)[:] = v
    print("simulating...")
    sim.simulate()
    r = {"hs": np.array(sim.tensor("hs")), "cT": np.array(sim.tensor("cT"))}

    hsd = np.asarray(r["hs"]).astype(np.float32)
    nslab = steps // SLAB
    hsd = hsd.reshape(128, nslab, SLAB, KT, BC)
    hsd = np.transpose(hsd, (4, 1, 2, 3, 0)).reshape(BC, steps, H)
    ctd = np.asarray(r["cT"]).astype(np.float32).reshape(128, KT, BC)
    ctd = np.transpose(ctd, (2, 1, 0)).reshape(BC, H)

    e_hs = exp_hs[:BC]
    err = np.abs(hsd - e_hs).max() / np.abs(e_hs).max()
    errc = np.abs(ctd - exp_c[:BC]).max() / np.abs(exp_c[:BC]).max()
    print(f"hs absmax rel err: {err:.4e}   cT: {errc:.4e}")
